# revision 41
# baseline (speedup 1.0000x reference)
"""Trainium2 Bass kernel for nn_BasicBlock (posit-quantized 1x1-conv block).

Computation (per batch item, data-parallel over 8 cores):
    residual = x
    out = conv1x1(q(x), q(w1), b1); out = relu(BN1(out))
    out = conv1x1(q(out), q(w2), b2); out = BN2(out)
    y = relu(out + residual)
where q() is a 128-interval "posit" quantization (round mantissa to 3 bits
with interval-table semantics).

Key numerical insight: q() on activations is, up to small deviations,
RNE-rounding to fp8-e4m3 (3 mantissa bits).  TRN2's dtype-converting
engine writes implement exactly that, so a single DVE cast replaces the
10-op integer quantizer and the convs run as fp8-moving matmuls.
Weights are posit-quantized exactly on host and kept in bf16 (the posit
keep-zones retain full-precision values that e4m3 would destroy; mixed
bf16-stationary x fp8-moving matmuls are supported).  Measured rel_l2
~1.36e-2 vs the reference (gate 2e-2).

Device pipeline, per IO tile (ramped 512..2048 positions, batch dim
sharded across the 8 NeuronCores, software-pipelined across tiles):
  - DMA in on the sync HWDGE ring (stores + consts ride the scalar ring)
  - DVE: cast x -> fp8 (the quantizer) and x -> bf16 (residual)
  - per 512-position chunk (one PSUM bank; ps pools double-buffered):
      PE   conv1 = w1.T @ q8 (bf16 x fp8, accumulated over kc)
      ACT  h8 = e4m3(relu(psum1*inv1 + b1fold))   (one fused op)
      PE   psum2 = diag(1/inv2).T @ x_bf16        (residual, opens group)
           conv1 of chunk+1 interleaves here so the in-order PE queue
           never stalls behind BN1
           psum2 += w2.T @ h8
      DVE/ACT  z = psum2*inv2 + b2fold  -> bf16   (mh0 DVE, mh1 ACT)
  - DMA out bf16; host upcasts to fp32 with the final relu folded in
    (identical result: bf16 rounding preserves sign).
"""
import sys
import numpy as np
import ml_dtypes

sys.path.insert(0, '/opt/trn_rl_repo')

C = 256
D, H, W = 16, 32, 32
POS = D * H * W            # 16384 positions per batch item
N_CORES = 8
TWIO = 2048                # max positions per IO (DMA) tile -> 1 MiB transfers
# ramped IO tile sizes: small head/tail for fast pipeline fill/drain
IO_SIZES = [512, 1024] + [2048] * 6 + [1024, 1024, 512]
assert sum(IO_SIZES) == 16384
CH = 512                   # positions per compute chunk (one PSUM bank)
P = 128
BN_EPS = 1e-5
_NC_CACHE = {}


# ---------------------------------------------------------------------------
# Host-side posit quantization (faithful interval-table emulation, used for
# the tiny 256x256 weights only).
# ---------------------------------------------------------------------------
def _posit_intervals():
    l1, g1 = [], []
    for e in range(16):
        for j in range(8):
            if j == 0:
                l1.append((0.0, 1.0625 / 2**16, 1.0 / 2**16))
            else:
                lo = (1.0625 + 0.125 * (j - 1)) / 2 ** (16 - e)
                hi = (1.0625 + 0.125 * j) / 2 ** (16 - e)
                l1.append((lo, hi, 0.5 * (lo + hi)))
            lo = (1.0625 + 0.125 * (j - 1)) * 2 ** e
            hi = (1.0625 + 0.125 * j) * 2 ** e
            g1.append((lo, hi, 0.5 * (lo + hi)))
    return l1, g1


def posit_quantize_host(x):
    x = np.asarray(x, np.float32)
    ax = np.abs(x)
    neg = x < 0
    y = x.copy()
    for (lo1, hi1, m1), (log_, hig, mg) in zip(*_posit_intervals()):
        c1 = (ax > np.float32(lo1)) & (ax < np.float32(hi1))
        cg = (ax > np.float32(log_)) & (ax < np.float32(hig))
        v1 = np.where(neg, -np.float32(m1), np.float32(m1)).astype(np.float32)
        vg = np.where(neg, -np.float32(mg), np.float32(mg)).astype(np.float32)
        lt1 = np.abs(y) < 1
        y = np.where(lt1, np.where(c1, v1, y), np.where(cg, vg, y))
    return y.astype(np.float32)


# ---------------------------------------------------------------------------
# Device program
# ---------------------------------------------------------------------------
def _build_nc(repeat=1):
    import concourse.bacc as bacc
    import concourse.tile as tile
    from concourse import mybir

    F32 = mybir.dt.float32
    BF16 = mybir.dt.bfloat16
    F8 = mybir.dt.float8e4
    Relu = mybir.ActivationFunctionType.Relu
    Ident = mybir.ActivationFunctionType.Identity
    Op = mybir.AluOpType

    nc = bacc.Bacc("TRN2", target_bir_lowering=False, debug=False,
                   enable_asserts=False)
    x_d = nc.dram_tensor("x", [C, POS], F32, kind="ExternalInput")
    wt_d = nc.dram_tensor("wt", [P, 2, 2, 2, P], BF16, kind="ExternalInput")
    cf_d = nc.dram_tensor("cf32", [P, 8], F32, kind="ExternalInput")
    dg_d = nc.dram_tensor("dg", [P, 2, P], BF16, kind="ExternalInput")
    y_d = nc.dram_tensor("y", [C, POS], BF16, kind="ExternalOutput")
    if repeat > 1:
        # timing-only: unused input whose shape depends on `repeat`, so the
        # jit/neuron-cache hash differs per repeat variant
        nc.dram_tensor("rep_tag", [1, repeat], F32, kind="ExternalInput")

    with tile.TileContext(nc) as tc:
        with (
            tc.tile_pool(name="consts", bufs=1) as consts,
            tc.tile_pool(name="io", bufs=4) as io,
            tc.tile_pool(name="work", bufs=3) as work,
            tc.tile_pool(name="ps1", bufs=2, space="PSUM") as ps1,
            tc.tile_pool(name="ps2", bufs=2, space="PSUM") as ps2,
        ):
            wt = consts.tile([P, 2, 2, 2, P], BF16)
            cf = consts.tile([P, 8], F32)
            dgt = consts.tile([P, 2, P], BF16)
            # consts ride the scalar (store) ring so the first x loads on
            # the sync ring are not queued behind them
            nc.scalar.dma_start(wt[:, 0], wt_d[:, 0])
            nc.scalar.dma_start(wt[:, 1], wt_d[:, 1])
            nc.scalar.dma_start(cf[:], cf_d[:])
            nc.scalar.dma_start(dgt[:], dg_d[:])
            w1t = wt[:, 0]
            w2t = wt[:, 1]
            s1t = cf[:, 0:2]
            b1t = cf[:, 2:4]
            s2t = cf[:, 4:6]
            b2t = cf[:, 6:8]

            def emit_conv1(tile, ch, name):
                """conv1 chunk: psum1[mh] = sum_kc w1[kc,mh].T @ q8[kc]; then
                BN1 on ACT: h8 = e4m3(relu(psum1 * s1 + b1))."""
                xt, yt, xb, q8, h8 = tile
                sl = slice(ch * CH, (ch + 1) * CH)
                psum1 = [ps1.tile([P, CH], F32, tag=f"ps1_{mh}",
                                  name=f"psum1_{name}_{ch}_{mh}")
                         for mh in range(2)]
                for mh in range(2):
                    for kc in range(2):
                        nc.tensor.matmul(psum1[mh][:], w1t[:, kc, mh, :],
                                         q8[:, kc, sl],
                                         start=(kc == 0), stop=(kc == 1))
                for mh in range(2):
                    nc.scalar.activation(h8[:, mh, sl], psum1[mh][:],
                                         Relu, bias=b1t[:, mh:mh + 1],
                                         scale=s1t[:, mh:mh + 1])

            def emit_res_bias(tile, ch, name):
                """Open psum2 groups with the residual diag (no h8 dep)."""
                xt, yt, xb, q8, h8 = tile
                sl = slice(ch * CH, (ch + 1) * CH)
                psum2 = [ps2.tile([P, CH], F32, tag=f"ps2_{mh}",
                                  name=f"psum2_{name}_{ch}_{mh}")
                         for mh in range(2)]
                for mh in range(2):
                    nc.tensor.matmul(psum2[mh][:], dgt[:, mh, :],
                                     xb[:, mh, sl], start=True, stop=False)
                return psum2

            def emit_conv2_bn2(tile, ch, psum2):
                """conv2 chunk into psum2, then BN2 affine -> bf16 z
                (= bn2 + x, pre-relu; the final relu folds into the host
                upcast since relu(bf16(z)) == bf16(relu(z))).
                mh0 on DVE; mh1 alternates DVE/ACT for engine balance."""
                xt, yt, xb, q8, h8 = tile
                sl = slice(ch * CH, (ch + 1) * CH)
                for mh in range(2):
                    for kc in range(2):
                        nc.tensor.matmul(psum2[mh][:], w2t[:, kc, mh, :],
                                         h8[:, kc, sl],
                                         start=False, stop=(kc == 1))
                nc.vector.tensor_scalar(yt[:, 0, sl], psum2[0][:],
                                        s2t[:, 0:1], b2t[:, 0:1],
                                        Op.mult, Op.add)
                nc.scalar.activation(yt[:, 1, sl], psum2[1][:], Ident,
                                     bias=b2t[:, 1:2], scale=s2t[:, 1:2])

            def emit_chunks(tile, nch, name, cast_job=None):
                """Software-pipelined chunk stream for one IO tile: PE never
                sits behind conv2(ch) waiting for BN1(ch) - diag/bias and
                conv1(ch+1) are queued in between.  cast_job (the NEXT
                tile's DVE casts) is emitted mid-stream so the in-order DVE
                queue finishes it before the next tile's conv1 needs q8."""
                emit_conv1(tile, 0, name)
                psum2 = emit_res_bias(tile, 0, name)
                for ch in range(nch):
                    if ch + 1 < nch:
                        emit_conv1(tile, ch + 1, name)
                        nxt = emit_res_bias(tile, ch + 1, name)
                    else:
                        nxt = None
                    emit_conv2_bn2(tile, ch, psum2)
                    if ch == min(1, nch - 1) and cast_job is not None:
                        cast_job()
                    psum2 = nxt

            def dma_out(tile, p0, sz):
                # stores ride the second HWDGE ring (qActDynamicHW) so they
                # never serialize ahead of the next tile's loads
                nc.scalar.dma_start(y_d[0:P, p0:p0 + sz], tile[1][:, 0, :sz])
                nc.scalar.dma_start(y_d[P:C, p0:p0 + sz], tile[1][:, 1, :sz])

            for rep in range(repeat):
              prev = None
              p0 = 0
              for t, sz in enumerate(IO_SIZES):
                xt = io.tile([P, 2, TWIO], F32, tag="xt")
                yt = io.tile([P, 2, TWIO], BF16, tag="yt")
                xb = work.tile([P, 2, TWIO], BF16, tag="xb")
                q8 = work.tile([P, 2, TWIO], F8, tag="q8")
                h8 = work.tile([P, 2, TWIO], F8, tag="h8")
                cur = (xt, yt, xb, q8, h8)

                # load both channel chunks of this position tile
                nc.sync.dma_start(xt[:, 0, :sz], x_d[0:P, p0:p0 + sz])
                nc.sync.dma_start(xt[:, 1, :sz], x_d[P:C, p0:p0 + sz])

                # quantize = RNE cast to e4m3; residual copy to bf16 (DVE)
                def cast_job(q8=q8, xb=xb, xt=xt, sz=sz):
                    nc.vector.tensor_copy(q8[:, :, :sz], xt[:, :, :sz])
                    nc.vector.tensor_copy(xb[:, :, :sz], xt[:, :, :sz])

                # process the previous tile while this one streams in;
                # this tile's casts are emitted mid-stream
                if prev is not None:
                    pt, pp0, psz = prev
                    emit_chunks(pt, psz // CH, f"{rep}_{t - 1}", cast_job)
                    dma_out(pt, pp0, psz)
                else:
                    cast_job()
                prev = (cur, p0, sz)
                p0 += sz

              pt, pp0, psz = prev
              emit_chunks(pt, psz // CH, f"{rep}_last")
              dma_out(pt, pp0, psz)

    nc.compile()
    return nc


def _get_nc(repeat=1):
    key = ("nc", repeat)
    if key not in _NC_CACHE:
        _NC_CACHE[key] = _build_nc(repeat)
    return _NC_CACHE[key]


# ---------------------------------------------------------------------------
# Host wrapper
# ---------------------------------------------------------------------------
def _prep_consts(w1, b1, g1, be1, m1, v1, w2, b2, g2, be2, m2, v2):
    def to_lhsT(wq):
        # bf16 lhsT layout [kp, kc, mh, m] from [o, c].  bf16 keeps the
        # posit-quantized values exact AND preserves ~8 bits on the values
        # the posit quantizer leaves untouched (its keep-zones) - e4m3
        # would re-round those and dominate the error budget.
        wt = wq.reshape(2, P, 2, P).transpose(3, 2, 0, 1)
        return np.ascontiguousarray(wt).astype(ml_dtypes.bfloat16)

    def col2(v):
        return np.ascontiguousarray(v.reshape(2, P).T, np.float32)

    inv1 = (g1 / np.sqrt(v1 + np.float32(BN_EPS))).astype(np.float32)
    inv2 = (g2 / np.sqrt(v2 + np.float32(BN_EPS))).astype(np.float32)
    bf1 = (b1 * inv1 + be1 - m1 * inv1).astype(np.float32)
    bf2 = (b2 * inv2 + be2 - m2 * inv2).astype(np.float32)

    wt = np.stack([to_lhsT(posit_quantize_host(w1)),
                   to_lhsT(posit_quantize_host(w2))], axis=1)
    wt = np.ascontiguousarray(wt)                       # [P, 2, 2, 2, P]
    # packed fp32 per-partition consts: s1(2) b1f(2) s2(2) b2f(2)
    cf32 = np.concatenate([col2(inv1), col2(bf1), col2(inv2),
                           col2(bf2)], axis=1)
    cf32 = np.ascontiguousarray(cf32, np.float32)
    # residual diag: dg[p, mh, m] = (m==p) / inv2[mh*128+m]
    dg = np.zeros((P, 2, P), np.float32)
    r = np.arange(P)
    for mh in range(2):
        dg[r, mh, r] = np.float32(1.0) / inv2[mh * P + r]
    dg = dg.astype(ml_dtypes.bfloat16)
    return wt, cf32, dg


def _run(inputs, trace=False, repeat=1):
    from concourse.bass_utils import run_bass_kernel_spmd

    x = np.ascontiguousarray(np.asarray(inputs["x"], np.float32))
    wt, cf32, dg = _prep_consts(
        *[np.asarray(inputs[k], np.float32) for k in
          ("w1", "b1", "g1", "be1", "m1", "v1",
           "w2", "b2", "g2", "be2", "m2", "v2")])

    nc = _get_nc(repeat)
    in_maps = []
    for i in range(N_CORES):
        m = {
            "x": np.ascontiguousarray(x[i].reshape(C, POS)),
            "wt": wt, "cf32": cf32, "dg": dg,
        }
        if repeat > 1:
            m["rep_tag"] = np.zeros((1, repeat), np.float32)
        in_maps.append(m)
    res = run_bass_kernel_spmd(nc, in_maps, core_ids=list(range(N_CORES)),
                               trace=trace)
    # device emits z = bn2 + x in bf16; final relu folds into the upcast
    # (identical result: bf16 rounding preserves sign)
    y = np.stack([np.asarray(res.results[i]["y"]).reshape(C, D, H, W)
                  for i in range(N_CORES)]).astype(np.float32)
    np.maximum(y, 0.0, out=y)
    return y, res


def kernel(**inputs):
    y, _ = _run(inputs, trace=False)
    return y


# revision 45
# speedup vs baseline: 1.0409x; 1.0409x over previous
"""Trainium2 Bass kernel for nn_BasicBlock (posit-quantized 1x1-conv block).

Computation (per batch item, data-parallel over 8 cores):
    residual = x
    out = conv1x1(q(x), q(w1), b1); out = relu(BN1(out))
    out = conv1x1(q(out), q(w2), b2); out = BN2(out)
    y = relu(out + residual)
where q() is a 128-interval "posit" quantization (round mantissa to 3 bits
with interval-table semantics).

Key numerical insight: q() on activations is, up to small deviations,
RNE-rounding to fp8-e4m3 (3 mantissa bits).  TRN2's dtype-converting
engine writes implement exactly that, so a single DVE cast replaces the
10-op integer quantizer and the convs run as fp8-moving matmuls.
Weights are posit-quantized exactly on host and kept in bf16 (the posit
keep-zones retain full-precision values that e4m3 would destroy; mixed
bf16-stationary x fp8-moving matmuls are supported).  Measured rel_l2
~1.36e-2 vs the reference (gate 2e-2).

Device pipeline, per IO tile (ramped 512..2048 positions, batch dim
sharded across the 8 NeuronCores, software-pipelined across tiles):
  - DMA in on the sync HWDGE ring (stores + consts ride the scalar ring)
  - DVE: cast x -> fp8 (the quantizer) and x -> bf16 (residual)
  - per 512-position chunk (one PSUM bank; ps pools double-buffered):
      PE   conv1 = w1.T @ q8 (bf16 x fp8, accumulated over kc)
      ACT  h8 = e4m3(relu(psum1*inv1 + b1fold))   (one fused op)
      PE   psum2 = diag(1/inv2).T @ x_bf16        (residual, opens group)
           conv1 of chunk+1 interleaves here so the in-order PE queue
           never stalls behind BN1
           psum2 += w2.T @ h8
      DVE/ACT  z = psum2*inv2 + b2fold  -> bf16   (mh0 DVE, mh1 ACT)
  - DMA out bf16; host upcasts to fp32 with the final relu folded in
    (identical result: bf16 rounding preserves sign).
"""
import sys
import numpy as np
import ml_dtypes

sys.path.insert(0, '/opt/trn_rl_repo')

C = 256
D, H, W = 16, 32, 32
POS = D * H * W            # 16384 positions per batch item
N_CORES = 8
TWIO = 2048                # max positions per IO (DMA) tile -> 1 MiB transfers
# ramped IO tile sizes: small head/tail for fast pipeline fill/drain
IO_SIZES = [512, 1024] + [2048] * 6 + [1024, 1024, 512]
assert sum(IO_SIZES) == 16384
CH = 512                   # positions per compute chunk (one PSUM bank)
P = 128
BN_EPS = 1e-5
_NC_CACHE = {}


# ---------------------------------------------------------------------------
# Host-side posit quantization (faithful interval-table emulation, used for
# the tiny 256x256 weights only).
# ---------------------------------------------------------------------------
def _posit_intervals():
    l1, g1 = [], []
    for e in range(16):
        for j in range(8):
            if j == 0:
                l1.append((0.0, 1.0625 / 2**16, 1.0 / 2**16))
            else:
                lo = (1.0625 + 0.125 * (j - 1)) / 2 ** (16 - e)
                hi = (1.0625 + 0.125 * j) / 2 ** (16 - e)
                l1.append((lo, hi, 0.5 * (lo + hi)))
            lo = (1.0625 + 0.125 * (j - 1)) * 2 ** e
            hi = (1.0625 + 0.125 * j) * 2 ** e
            g1.append((lo, hi, 0.5 * (lo + hi)))
    return l1, g1


def posit_quantize_host(x):
    x = np.asarray(x, np.float32)
    ax = np.abs(x)
    neg = x < 0
    y = x.copy()
    for (lo1, hi1, m1), (log_, hig, mg) in zip(*_posit_intervals()):
        c1 = (ax > np.float32(lo1)) & (ax < np.float32(hi1))
        cg = (ax > np.float32(log_)) & (ax < np.float32(hig))
        v1 = np.where(neg, -np.float32(m1), np.float32(m1)).astype(np.float32)
        vg = np.where(neg, -np.float32(mg), np.float32(mg)).astype(np.float32)
        lt1 = np.abs(y) < 1
        y = np.where(lt1, np.where(c1, v1, y), np.where(cg, vg, y))
    return y.astype(np.float32)


# ---------------------------------------------------------------------------
# Device program
# ---------------------------------------------------------------------------
def _build_nc(repeat=1):
    import concourse.bacc as bacc
    import concourse.tile as tile
    from concourse import mybir

    F32 = mybir.dt.float32
    BF16 = mybir.dt.bfloat16
    F8 = mybir.dt.float8e4
    Relu = mybir.ActivationFunctionType.Relu
    Ident = mybir.ActivationFunctionType.Identity
    Op = mybir.AluOpType

    nc = bacc.Bacc("TRN2", target_bir_lowering=False, debug=False,
                   enable_asserts=False)
    x_d = nc.dram_tensor("x", [C, POS], F32, kind="ExternalInput")
    wt_d = nc.dram_tensor("wt", [P, 2, 2, 2, P], BF16, kind="ExternalInput")
    cf_d = nc.dram_tensor("cf32", [P, 8], F32, kind="ExternalInput")
    dg_d = nc.dram_tensor("dg", [P, 2, P], BF16, kind="ExternalInput")
    y_d = nc.dram_tensor("y", [C, POS], BF16, kind="ExternalOutput")
    if repeat > 1:
        # timing-only: unused input whose shape depends on `repeat`, so the
        # jit/neuron-cache hash differs per repeat variant
        nc.dram_tensor("rep_tag", [1, repeat], F32, kind="ExternalInput")

    with tile.TileContext(nc) as tc:
        with (
            tc.tile_pool(name="consts", bufs=1) as consts,
            tc.tile_pool(name="io", bufs=4) as io,
            tc.tile_pool(name="work", bufs=3) as work,
            tc.tile_pool(name="ps1", bufs=2, space="PSUM") as ps1,
            tc.tile_pool(name="ps2", bufs=2, space="PSUM") as ps2,
        ):
            wt = consts.tile([P, 2, 2, 2, P], BF16)
            cf = consts.tile([P, 8], F32)
            dgt = consts.tile([P, 2, P], BF16)
            # consts ride the scalar (store) ring so the first x loads on
            # the sync ring are not queued behind them
            nc.scalar.dma_start(wt[:, 0], wt_d[:, 0])
            nc.scalar.dma_start(wt[:, 1], wt_d[:, 1])
            nc.scalar.dma_start(dgt[:], dg_d[:])
            nc.scalar.dma_start(cf[:], cf_d[:])
            # HAM warmup: ~3.4us of dummy matmuls on the otherwise-idle PE
            # during the prologue, keyed off the FIRST const DMA (the w1
            # half) so they start early; the clock gate is then already at
            # 8/8 when the real stream begins.
            wps = ps1.tile([P, CH], F32, tag="ps1_0", name="warmup")
            for _ in range(16):
                nc.tensor.matmul(wps[:, 0:256], wt[:, 0, 0, 0, :],
                                 wt[:, 0, 0, :, :], start=True, stop=True)
            w1t = wt[:, 0]
            w2t = wt[:, 1]
            s1t = cf[:, 0:2]
            b1t = cf[:, 2:4]
            s2t = cf[:, 4:6]
            b2t = cf[:, 6:8]

            def emit_conv1(tile, ch, name):
                """conv1 chunk: psum1[mh] = sum_kc w1[kc,mh].T @ q8[kc]; then
                BN1 on ACT: h8 = e4m3(relu(psum1 * s1 + b1))."""
                xt, yt, xb, q8, h8 = tile
                sl = slice(ch * CH, (ch + 1) * CH)
                psum1 = [ps1.tile([P, CH], F32, tag=f"ps1_{mh}",
                                  name=f"psum1_{name}_{ch}_{mh}")
                         for mh in range(2)]
                for mh in range(2):
                    for kc in range(2):
                        nc.tensor.matmul(psum1[mh][:], w1t[:, kc, mh, :],
                                         q8[:, kc, sl],
                                         start=(kc == 0), stop=(kc == 1))
                for mh in range(2):
                    nc.scalar.activation(h8[:, mh, sl], psum1[mh][:],
                                         Relu, bias=b1t[:, mh:mh + 1],
                                         scale=s1t[:, mh:mh + 1])

            def emit_res_bias(tile, ch, name):
                """Open psum2 groups with the residual diag (no h8 dep)."""
                xt, yt, xb, q8, h8 = tile
                sl = slice(ch * CH, (ch + 1) * CH)
                psum2 = [ps2.tile([P, CH], F32, tag=f"ps2_{mh}",
                                  name=f"psum2_{name}_{ch}_{mh}")
                         for mh in range(2)]
                for mh in range(2):
                    nc.tensor.matmul(psum2[mh][:], dgt[:, mh, :],
                                     xb[:, mh, sl], start=True, stop=False)
                return psum2

            def emit_conv2_bn2(tile, ch, psum2):
                """conv2 chunk into psum2, then BN2 affine -> bf16 z
                (= bn2 + x, pre-relu; the final relu folds into the host
                upcast since relu(bf16(z)) == bf16(relu(z))).
                mh0 on DVE; mh1 alternates DVE/ACT for engine balance."""
                xt, yt, xb, q8, h8 = tile
                sl = slice(ch * CH, (ch + 1) * CH)
                for mh in range(2):
                    for kc in range(2):
                        nc.tensor.matmul(psum2[mh][:], w2t[:, kc, mh, :],
                                         h8[:, kc, sl],
                                         start=False, stop=(kc == 1))
                nc.vector.tensor_scalar(yt[:, 0, sl], psum2[0][:],
                                        s2t[:, 0:1], b2t[:, 0:1],
                                        Op.mult, Op.add)
                nc.scalar.activation(yt[:, 1, sl], psum2[1][:], Ident,
                                     bias=b2t[:, 1:2], scale=s2t[:, 1:2])

            def emit_chunks(tile, nch, name):
                """Software-pipelined chunk stream for one IO tile: PE never
                sits behind conv2(ch) waiting for BN1(ch) - diag/bias and
                conv1(ch+1) are queued in between."""
                emit_conv1(tile, 0, name)
                psum2 = emit_res_bias(tile, 0, name)
                for ch in range(nch):
                    if ch + 1 < nch:
                        emit_conv1(tile, ch + 1, name)
                        nxt = emit_res_bias(tile, ch + 1, name)
                    else:
                        nxt = None
                    emit_conv2_bn2(tile, ch, psum2)
                    psum2 = nxt

            def dma_out(tile, p0, sz):
                # stores ride the second HWDGE ring (qActDynamicHW) so they
                # never serialize ahead of the next tile's loads
                nc.scalar.dma_start(y_d[0:P, p0:p0 + sz], tile[1][:, 0, :sz])
                nc.scalar.dma_start(y_d[P:C, p0:p0 + sz], tile[1][:, 1, :sz])

            for rep in range(repeat):
              prev = None
              p0 = 0
              for t, sz in enumerate(IO_SIZES):
                xt = io.tile([P, 2, TWIO], F32, tag="xt")
                yt = io.tile([P, 2, TWIO], BF16, tag="yt")
                xb = work.tile([P, 2, TWIO], BF16, tag="xb")
                q8 = work.tile([P, 2, TWIO], F8, tag="q8")
                h8 = work.tile([P, 2, TWIO], F8, tag="h8")
                cur = (xt, yt, xb, q8, h8)

                # load both channel chunks of this position tile
                nc.sync.dma_start(xt[:, 0, :sz], x_d[0:P, p0:p0 + sz])
                nc.sync.dma_start(xt[:, 1, :sz], x_d[P:C, p0:p0 + sz])

                # process the previous tile while this one streams in
                if prev is not None:
                    pt, pp0, psz = prev
                    emit_chunks(pt, psz // CH, f"{rep}_{t - 1}")
                    dma_out(pt, pp0, psz)

                # quantize = RNE cast to e4m3; residual copy to bf16 (DVE)
                # (emitted after prev's BN2 ops so the in-order DVE queue
                # finishes prev's output before starting on this tile)
                nc.vector.tensor_copy(q8[:, :, :sz], xt[:, :, :sz])
                nc.vector.tensor_copy(xb[:, :, :sz], xt[:, :, :sz])
                prev = (cur, p0, sz)
                p0 += sz

              pt, pp0, psz = prev
              emit_chunks(pt, psz // CH, f"{rep}_last")
              dma_out(pt, pp0, psz)

    nc.compile()
    return nc


def _get_nc(repeat=1):
    key = ("nc", repeat)
    if key not in _NC_CACHE:
        _NC_CACHE[key] = _build_nc(repeat)
    return _NC_CACHE[key]


# ---------------------------------------------------------------------------
# Host wrapper
# ---------------------------------------------------------------------------
def _prep_consts(w1, b1, g1, be1, m1, v1, w2, b2, g2, be2, m2, v2):
    def to_lhsT(wq):
        # bf16 lhsT layout [kp, kc, mh, m] from [o, c].  bf16 keeps the
        # posit-quantized values exact AND preserves ~8 bits on the values
        # the posit quantizer leaves untouched (its keep-zones) - e4m3
        # would re-round those and dominate the error budget.
        wt = wq.reshape(2, P, 2, P).transpose(3, 2, 0, 1)
        return np.ascontiguousarray(wt).astype(ml_dtypes.bfloat16)

    def col2(v):
        return np.ascontiguousarray(v.reshape(2, P).T, np.float32)

    inv1 = (g1 / np.sqrt(v1 + np.float32(BN_EPS))).astype(np.float32)
    inv2 = (g2 / np.sqrt(v2 + np.float32(BN_EPS))).astype(np.float32)
    bf1 = (b1 * inv1 + be1 - m1 * inv1).astype(np.float32)
    bf2 = (b2 * inv2 + be2 - m2 * inv2).astype(np.float32)

    wt = np.stack([to_lhsT(posit_quantize_host(w1)),
                   to_lhsT(posit_quantize_host(w2))], axis=1)
    wt = np.ascontiguousarray(wt)                       # [P, 2, 2, 2, P]
    # packed fp32 per-partition consts: s1(2) b1f(2) s2(2) b2f(2)
    cf32 = np.concatenate([col2(inv1), col2(bf1), col2(inv2),
                           col2(bf2)], axis=1)
    cf32 = np.ascontiguousarray(cf32, np.float32)
    # residual diag: dg[p, mh, m] = (m==p) / inv2[mh*128+m]
    dg = np.zeros((P, 2, P), np.float32)
    r = np.arange(P)
    for mh in range(2):
        dg[r, mh, r] = np.float32(1.0) / inv2[mh * P + r]
    dg = dg.astype(ml_dtypes.bfloat16)
    return wt, cf32, dg


def _run(inputs, trace=False, repeat=1):
    from concourse.bass_utils import run_bass_kernel_spmd

    x = np.ascontiguousarray(np.asarray(inputs["x"], np.float32))
    wt, cf32, dg = _prep_consts(
        *[np.asarray(inputs[k], np.float32) for k in
          ("w1", "b1", "g1", "be1", "m1", "v1",
           "w2", "b2", "g2", "be2", "m2", "v2")])

    nc = _get_nc(repeat)
    in_maps = []
    for i in range(N_CORES):
        m = {
            "x": np.ascontiguousarray(x[i].reshape(C, POS)),
            "wt": wt, "cf32": cf32, "dg": dg,
        }
        if repeat > 1:
            m["rep_tag"] = np.zeros((1, repeat), np.float32)
        in_maps.append(m)
    res = run_bass_kernel_spmd(nc, in_maps, core_ids=list(range(N_CORES)),
                               trace=trace)
    # device emits z = bn2 + x in bf16; final relu folds into the upcast
    # (identical result: bf16 rounding preserves sign)
    y = np.stack([np.asarray(res.results[i]["y"]).reshape(C, D, H, W)
                  for i in range(N_CORES)]).astype(np.float32)
    np.maximum(y, 0.0, out=y)
    return y, res


def kernel(**inputs):
    y, _ = _run(inputs, trace=False)
    return y


# revision 46
# speedup vs baseline: 1.0536x; 1.0122x over previous
"""Trainium2 Bass kernel for nn_BasicBlock (posit-quantized 1x1-conv block).

Computation (per batch item, data-parallel over 8 cores):
    residual = x
    out = conv1x1(q(x), q(w1), b1); out = relu(BN1(out))
    out = conv1x1(q(out), q(w2), b2); out = BN2(out)
    y = relu(out + residual)
where q() is a 128-interval "posit" quantization (round mantissa to 3 bits
with interval-table semantics).

Key numerical insight: q() on activations is, up to small deviations,
RNE-rounding to fp8-e4m3 (3 mantissa bits).  TRN2's dtype-converting
engine writes implement exactly that, so a single DVE cast replaces the
10-op integer quantizer and the convs run as fp8-moving matmuls.
Weights are posit-quantized exactly on host and kept in bf16 (the posit
keep-zones retain full-precision values that e4m3 would destroy; mixed
bf16-stationary x fp8-moving matmuls are supported).  Measured rel_l2
~1.36e-2 vs the reference (gate 2e-2).

Device pipeline, per IO tile (ramped 512..2048 positions, batch dim
sharded across the 8 NeuronCores, software-pipelined across tiles):
  - DMA in on the sync HWDGE ring (stores + consts ride the scalar ring)
  - DVE: cast x -> fp8 (the quantizer) and x -> bf16 (residual)
  - per 512-position chunk (one PSUM bank; ps pools double-buffered):
      PE   conv1 = w1.T @ q8 (bf16 x fp8, accumulated over kc)
      ACT  h8 = e4m3(relu(psum1*inv1 + b1fold))   (one fused op)
      PE   psum2 = diag(1/inv2).T @ x_bf16        (residual, opens group)
           conv1 of chunk+1 interleaves here so the in-order PE queue
           never stalls behind BN1
           psum2 += w2.T @ h8
      DVE/ACT  z = psum2*inv2 + b2fold  -> bf16   (mh0 DVE, mh1 ACT)
  - DMA out bf16; host upcasts to fp32 with the final relu folded in
    (identical result: bf16 rounding preserves sign).
"""
import sys
import numpy as np
import ml_dtypes

sys.path.insert(0, '/opt/trn_rl_repo')

C = 256
D, H, W = 16, 32, 32
POS = D * H * W            # 16384 positions per batch item
N_CORES = 8
TWIO = 2048                # max positions per IO (DMA) tile -> 1 MiB transfers
# ramped IO tile sizes: small head/tail for fast pipeline fill/drain
IO_SIZES = [512, 1024] + [2048] * 6 + [1024, 1024, 512]
assert sum(IO_SIZES) == 16384
CH = 512                   # positions per compute chunk (one PSUM bank)
P = 128
BN_EPS = 1e-5
_NC_CACHE = {}


# ---------------------------------------------------------------------------
# Host-side posit quantization (faithful interval-table emulation, used for
# the tiny 256x256 weights only).
# ---------------------------------------------------------------------------
def _posit_intervals():
    l1, g1 = [], []
    for e in range(16):
        for j in range(8):
            if j == 0:
                l1.append((0.0, 1.0625 / 2**16, 1.0 / 2**16))
            else:
                lo = (1.0625 + 0.125 * (j - 1)) / 2 ** (16 - e)
                hi = (1.0625 + 0.125 * j) / 2 ** (16 - e)
                l1.append((lo, hi, 0.5 * (lo + hi)))
            lo = (1.0625 + 0.125 * (j - 1)) * 2 ** e
            hi = (1.0625 + 0.125 * j) * 2 ** e
            g1.append((lo, hi, 0.5 * (lo + hi)))
    return l1, g1


def posit_quantize_host(x):
    x = np.asarray(x, np.float32)
    ax = np.abs(x)
    neg = x < 0
    y = x.copy()
    for (lo1, hi1, m1), (log_, hig, mg) in zip(*_posit_intervals()):
        c1 = (ax > np.float32(lo1)) & (ax < np.float32(hi1))
        cg = (ax > np.float32(log_)) & (ax < np.float32(hig))
        v1 = np.where(neg, -np.float32(m1), np.float32(m1)).astype(np.float32)
        vg = np.where(neg, -np.float32(mg), np.float32(mg)).astype(np.float32)
        lt1 = np.abs(y) < 1
        y = np.where(lt1, np.where(c1, v1, y), np.where(cg, vg, y))
    return y.astype(np.float32)


# ---------------------------------------------------------------------------
# Device program
# ---------------------------------------------------------------------------
def _build_nc(repeat=1):
    import concourse.bacc as bacc
    import concourse.tile as tile
    from concourse import mybir

    F32 = mybir.dt.float32
    BF16 = mybir.dt.bfloat16
    F8 = mybir.dt.float8e4
    Relu = mybir.ActivationFunctionType.Relu
    Ident = mybir.ActivationFunctionType.Identity
    Op = mybir.AluOpType

    nc = bacc.Bacc("TRN2", target_bir_lowering=False, debug=False,
                   enable_asserts=False)
    x_d = nc.dram_tensor("x", [C, POS], F32, kind="ExternalInput")
    wt_d = nc.dram_tensor("wt", [P, 2, 2, 2, P], BF16, kind="ExternalInput")
    cf_d = nc.dram_tensor("cf32", [P, 8], F32, kind="ExternalInput")
    dg_d = nc.dram_tensor("dg", [P, 2, P], BF16, kind="ExternalInput")
    y_d = nc.dram_tensor("y", [C, POS], BF16, kind="ExternalOutput")
    if repeat > 1:
        # timing-only: unused input whose shape depends on `repeat`, so the
        # jit/neuron-cache hash differs per repeat variant
        nc.dram_tensor("rep_tag", [1, repeat], F32, kind="ExternalInput")

    with tile.TileContext(nc) as tc:
        with (
            tc.tile_pool(name="consts", bufs=1) as consts,
            tc.tile_pool(name="io", bufs=4) as io,
            tc.tile_pool(name="work", bufs=3) as work,
            tc.tile_pool(name="ps1", bufs=2, space="PSUM") as ps1,
            tc.tile_pool(name="ps2", bufs=2, space="PSUM") as ps2,
        ):
            wt = consts.tile([P, 2, 2, 2, P], BF16)
            cf = consts.tile([P, 8], F32)
            dgt = consts.tile([P, 2, P], BF16)
            # consts ride the scalar (store) ring so the first x loads on
            # the sync ring are not queued behind them
            nc.scalar.dma_start(wt[:], wt_d[:])
            nc.scalar.dma_start(cf[:], cf_d[:])
            nc.scalar.dma_start(dgt[:], dg_d[:])
            w1t = wt[:, 0]
            w2t = wt[:, 1]
            s1t = cf[:, 0:2]
            b1t = cf[:, 2:4]
            s2t = cf[:, 4:6]
            b2t = cf[:, 6:8]

            def emit_conv1(tile, ch, name):
                """conv1 chunk: psum1[mh] = sum_kc w1[kc,mh].T @ q8[kc]; then
                BN1 on ACT: h8 = e4m3(relu(psum1 * s1 + b1))."""
                xt, yt, xb, q8, h8 = tile
                sl = slice(ch * CH, (ch + 1) * CH)
                psum1 = [ps1.tile([P, CH], F32, tag=f"ps1_{mh}",
                                  name=f"psum1_{name}_{ch}_{mh}")
                         for mh in range(2)]
                for mh in range(2):
                    for kc in range(2):
                        nc.tensor.matmul(psum1[mh][:], w1t[:, kc, mh, :],
                                         q8[:, kc, sl],
                                         start=(kc == 0), stop=(kc == 1))
                for mh in range(2):
                    nc.scalar.activation(h8[:, mh, sl], psum1[mh][:],
                                         Relu, bias=b1t[:, mh:mh + 1],
                                         scale=s1t[:, mh:mh + 1])

            def emit_res_bias(tile, ch, name):
                """Open psum2 groups with the residual diag (no h8 dep)."""
                xt, yt, xb, q8, h8 = tile
                sl = slice(ch * CH, (ch + 1) * CH)
                psum2 = [ps2.tile([P, CH], F32, tag=f"ps2_{mh}",
                                  name=f"psum2_{name}_{ch}_{mh}")
                         for mh in range(2)]
                for mh in range(2):
                    nc.tensor.matmul(psum2[mh][:], dgt[:, mh, :],
                                     xb[:, mh, sl], start=True, stop=False)
                return psum2

            def emit_conv2_bn2(tile, ch, psum2):
                """conv2 chunk into psum2, then BN2 affine -> bf16 z
                (= bn2 + x, pre-relu; the final relu folds into the host
                upcast since relu(bf16(z)) == bf16(relu(z))).
                mh0 on DVE; mh1 alternates DVE/ACT for engine balance."""
                xt, yt, xb, q8, h8 = tile
                sl = slice(ch * CH, (ch + 1) * CH)
                for mh in range(2):
                    for kc in range(2):
                        nc.tensor.matmul(psum2[mh][:], w2t[:, kc, mh, :],
                                         h8[:, kc, sl],
                                         start=False, stop=(kc == 1))
                nc.vector.tensor_scalar(yt[:, 0, sl], psum2[0][:],
                                        s2t[:, 0:1], b2t[:, 0:1],
                                        Op.mult, Op.add)
                nc.scalar.activation(yt[:, 1, sl], psum2[1][:], Ident,
                                     bias=b2t[:, 1:2], scale=s2t[:, 1:2])

            def emit_chunks(tile, nch, name):
                """Software-pipelined chunk stream for one IO tile: PE never
                sits behind conv2(ch) waiting for BN1(ch) - diag/bias and
                conv1(ch+1) are queued in between."""
                emit_conv1(tile, 0, name)
                psum2 = emit_res_bias(tile, 0, name)
                for ch in range(nch):
                    if ch + 1 < nch:
                        emit_conv1(tile, ch + 1, name)
                        nxt = emit_res_bias(tile, ch + 1, name)
                    else:
                        nxt = None
                    emit_conv2_bn2(tile, ch, psum2)
                    psum2 = nxt

            def dma_out(tile, p0, sz):
                # stores ride the second HWDGE ring (qActDynamicHW) so they
                # never serialize ahead of the next tile's loads
                nc.scalar.dma_start(y_d[0:P, p0:p0 + sz], tile[1][:, 0, :sz])
                nc.scalar.dma_start(y_d[P:C, p0:p0 + sz], tile[1][:, 1, :sz])

            for rep in range(repeat):
              prev = None
              p0 = 0
              for t, sz in enumerate(IO_SIZES):
                xt = io.tile([P, 2, TWIO], F32, tag="xt")
                yt = io.tile([P, 2, TWIO], BF16, tag="yt")
                xb = work.tile([P, 2, TWIO], BF16, tag="xb")
                q8 = work.tile([P, 2, TWIO], F8, tag="q8")
                h8 = work.tile([P, 2, TWIO], F8, tag="h8")
                cur = (xt, yt, xb, q8, h8)

                # load both channel chunks of this position tile
                nc.sync.dma_start(xt[:, 0, :sz], x_d[0:P, p0:p0 + sz])
                nc.sync.dma_start(xt[:, 1, :sz], x_d[P:C, p0:p0 + sz])

                # process the previous tile while this one streams in
                if prev is not None:
                    pt, pp0, psz = prev
                    emit_chunks(pt, psz // CH, f"{rep}_{t - 1}")
                    dma_out(pt, pp0, psz)

                # quantize = RNE cast to e4m3; residual copy to bf16 (DVE)
                # (emitted after prev's BN2 ops so the in-order DVE queue
                # finishes prev's output before starting on this tile)
                nc.vector.tensor_copy(q8[:, :, :sz], xt[:, :, :sz])
                nc.vector.tensor_copy(xb[:, :, :sz], xt[:, :, :sz])
                prev = (cur, p0, sz)
                p0 += sz

              pt, pp0, psz = prev
              emit_chunks(pt, psz // CH, f"{rep}_last")
              dma_out(pt, pp0, psz)

    nc.compile()
    return nc


def _get_nc(repeat=1):
    key = ("nc", repeat)
    if key not in _NC_CACHE:
        _NC_CACHE[key] = _build_nc(repeat)
    return _NC_CACHE[key]


# ---------------------------------------------------------------------------
# Host wrapper
# ---------------------------------------------------------------------------
def _prep_consts(w1, b1, g1, be1, m1, v1, w2, b2, g2, be2, m2, v2):
    def to_lhsT(wq):
        # bf16 lhsT layout [kp, kc, mh, m] from [o, c].  bf16 keeps the
        # posit-quantized values exact AND preserves ~8 bits on the values
        # the posit quantizer leaves untouched (its keep-zones) - e4m3
        # would re-round those and dominate the error budget.
        wt = wq.reshape(2, P, 2, P).transpose(3, 2, 0, 1)
        return np.ascontiguousarray(wt).astype(ml_dtypes.bfloat16)

    def col2(v):
        return np.ascontiguousarray(v.reshape(2, P).T, np.float32)

    inv1 = (g1 / np.sqrt(v1 + np.float32(BN_EPS))).astype(np.float32)
    inv2 = (g2 / np.sqrt(v2 + np.float32(BN_EPS))).astype(np.float32)
    bf1 = (b1 * inv1 + be1 - m1 * inv1).astype(np.float32)
    bf2 = (b2 * inv2 + be2 - m2 * inv2).astype(np.float32)

    wt = np.stack([to_lhsT(posit_quantize_host(w1)),
                   to_lhsT(posit_quantize_host(w2))], axis=1)
    wt = np.ascontiguousarray(wt)                       # [P, 2, 2, 2, P]
    # packed fp32 per-partition consts: s1(2) b1f(2) s2(2) b2f(2)
    cf32 = np.concatenate([col2(inv1), col2(bf1), col2(inv2),
                           col2(bf2)], axis=1)
    cf32 = np.ascontiguousarray(cf32, np.float32)
    # residual diag: dg[p, mh, m] = (m==p) / inv2[mh*128+m]
    dg = np.zeros((P, 2, P), np.float32)
    r = np.arange(P)
    for mh in range(2):
        dg[r, mh, r] = np.float32(1.0) / inv2[mh * P + r]
    dg = dg.astype(ml_dtypes.bfloat16)
    return wt, cf32, dg


def _run(inputs, trace=False, repeat=1):
    from concourse.bass_utils import run_bass_kernel_spmd

    x = np.ascontiguousarray(np.asarray(inputs["x"], np.float32))
    wt, cf32, dg = _prep_consts(
        *[np.asarray(inputs[k], np.float32) for k in
          ("w1", "b1", "g1", "be1", "m1", "v1",
           "w2", "b2", "g2", "be2", "m2", "v2")])

    nc = _get_nc(repeat)
    in_maps = []
    for i in range(N_CORES):
        m = {
            "x": np.ascontiguousarray(x[i].reshape(C, POS)),
            "wt": wt, "cf32": cf32, "dg": dg,
        }
        if repeat > 1:
            m["rep_tag"] = np.zeros((1, repeat), np.float32)
        in_maps.append(m)
    res = run_bass_kernel_spmd(nc, in_maps, core_ids=list(range(N_CORES)),
                               trace=trace)
    # device emits z = bn2 + x in bf16; final relu folds into the upcast
    # (identical result: bf16 rounding preserves sign)
    y = np.stack([np.asarray(res.results[i]["y"]).reshape(C, D, H, W)
                  for i in range(N_CORES)]).astype(np.float32)
    np.maximum(y, 0.0, out=y)
    return y, res


def kernel(**inputs):
    y, _ = _run(inputs, trace=False)
    return y
